# revision 47
# baseline (speedup 1.0000x reference)
"""Tensor-parallel (over GQA head groups) multi-head attention for 8 trn2 cores.

Each core owns 4 query heads + their shared kv head (one GQA group), the
matching 384 rows of wqkv and 256 columns of wo.  Every core computes a full
[S, D] partial of the output projection; the host sums the 8 partials.

Dataflow inside one core (fp32 data, float32r PE mode for all big matmuls --
same bytes, 4x the fp32 matmul rate at ~3e-4 relative error):
  qkvT [384, S] = wqkvT_local.T @ hT          (contraction dim on partitions)
  rope(qT, kT)  = raw*cos + pairswap(raw)*sin (pair swap via the DVE 32-lane
                                               stream_shuffle; rotation sign
                                               folded into the sin table,
                                               1/sqrt(hd) into the q tables)
  sT [ks, qs]   = kT.T-slices @ qT            (scores transposed so the
                                               softmax denominator can come
                                               from a matmul: even/odd heads
                                               at base partitions 0/64 run as
                                               concurrent row-tiled matmuls)
  exp on ScalarE straight out of PSUM, no max-subtraction (scores ~N(0,1));
  causal masking = skip fully-masked column ranges + one triangle multiply
  per diagonal block.
  out2T [65, qs] += v_aug.T @ exp             (ones column appended to V:
                                               row 64 = softmax denominator)
  o2 = out2T[0:64] * bcast(1/out2T[64])       (broadcast via ones-column
                                               matmul; reciprocal on 64 lanes)
  partial [qs, e] = o2-slices.T @ woT_local   (accumulate 2 contraction chunks)
"""

import sys

if "/opt/trn_rl_repo" not in sys.path:
    sys.path.insert(0, "/opt/trn_rl_repo")

import numpy as np

S = 2048
D = 2048
HD = 64
N_HEAD = 32
N_KV = 8
NCORES = 8
QH_PER_CORE = N_HEAD // NCORES  # 4
KV_SIZE = N_KV * HD  # 512

_CACHE = {}


def _build_module():
    from contextlib import ExitStack

    import concourse.mybir as mybir
    import concourse.tile as tile
    from concourse import bacc
    from concourse.bass import ds

    FP = mybir.dt.float32
    FPR = mybir.dt.float32r  # fp32 bits, single-pass PE mode: 4x matmul rate
    EXP = mybir.ActivationFunctionType.Exp

    def f32r(ap):
        return ap.bitcast(FPR)

    nc = bacc.Bacc(
        "TRN2",
        target_bir_lowering=False,
        debug=False,
        enable_asserts=False,
        num_devices=NCORES,
    )

    # [p, 2048*g + s] = hidden[s, 128*g + p]
    hT = nc.dram_tensor("hT", [128, 16 * S], FPR, kind="ExternalInput").ap()
    # [p, 384*g + r] = wqkv_local[r, 128*g + p]; r: 0-255 q, 256-319 k, 320-383 v
    wq = nc.dram_tensor("wq", [128, 16 * 384], FPR, kind="ExternalInput").ap()
    # [p, 2048*g + e] = wo[e, 256*core + 128*g + p]
    wo = nc.dram_tensor("wo", [128, 2 * 2048], FPR, kind="ExternalInput").ap()
    # [p, s] cos (cols 0:2048) | sin' (cols 2048:4096), two head copies, /8
    rq = nc.dram_tensor("rq", [128, 2 * S], FP, kind="ExternalInput").ap()
    # cols 0:128 tri[p, f] = (p <= f); cols 128:256 identity[p, f] = (p == f)
    tri = nc.dram_tensor("tri", [128, 256], FPR, kind="ExternalInput").ap()
    out = nc.dram_tensor("out", [S, D], FP, kind="ExternalOutput").ap()

    with tile.TileContext(nc) as tc, ExitStack() as ctx:
        const = ctx.enter_context(tc.tile_pool(name="const", bufs=1))
        # wq as 4 separate tiles; chunk 0 DMAs now, chunks 1-3 interleave
        # into the first g-loop so the opening hT chunks aren't starved
        wqt = [
            const.tile([128, 4 * 384], FP, tag=f"wq{k}", name=f"wq_sb{k}")
            for k in range(4)
        ]
        nc.sync.dma_start(f32r(wqt[0][:]), wq[:, ds(0, 4 * 384)])

        def wq_ap(g, m):
            return wqt[g // 4][:, ds(384 * (g % 4) + 128 * m, 128)]

        wo_sb = const.tile([128, 4096], FP, tag="wo")
        rq_sb = const.tile([128, 4096], FP, tag="rq")
        tri_sb = const.tile([128, 256], FP, tag="tri")
        # identity slice at partitions 64-127: the v-transpose matmul's
        # operands then share base_partition 64 (v lives in kv[64:128])
        ident64 = tri_sb[64:128, 192:256]
        # all-ones row hiding inside the triangle mask: tri[64, 64:128] == 1
        ones_row = tri_sb[64:65, 64:128]

        qraw = ctx.enter_context(tc.tile_pool(name="qraw", bufs=1))
        q01 = qraw.tile([128, S], FP, tag="q01")
        q23 = qraw.tile([128, S], FP, tag="q23")
        kv = qraw.tile([128, S], FP, tag="kv")
        qsw = ctx.enter_context(tc.tile_pool(name="qsw", bufs=1))
        q01s = qsw.tile([128, S], FP, tag="q01s")
        q23s = qsw.tile([128, S], FP, tag="q23s")
        ks = qsw.tile([128, S], FP, tag="ks")
        vpool = ctx.enter_context(tc.tile_pool(name="vsb", bufs=1))
        v_sb = vpool.tile([128, 16 * 65], FP, tag="v")
        o2pool = ctx.enter_context(tc.tile_pool(name="o2", bufs=1))
        o2a = o2pool.tile([128, S], FP, tag="o2a")
        o2b = o2pool.tile([128, S], FP, tag="o2b")

        outpool = ctx.enter_context(tc.tile_pool(name="ost", bufs=4))

        phase1_ctx = ExitStack()
        hpool = phase1_ctx.enter_context(tc.tile_pool(name="hp", bufs=6))
        scpool = phase1_ctx.enter_context(tc.tile_pool(name="sc", bufs=2))
        psA = phase1_ctx.enter_context(tc.tile_pool(name="psA", bufs=6, space="PSUM"))
        psT = phase1_ctx.enter_context(tc.tile_pool(name="psT", bufs=2, space="PSUM"))

        # ---- phase 1 + 1.5: qkvT projection, rope + v transpose --------
        # interleaved per 1024-wide column half so attention on qs-chunks
        # 0-1 can start while the second half is still projecting
        SWAP_MASK = [i ^ 1 for i in range(32)]
        MUL = mybir.AluOpType.mult
        v65 = v_sb.rearrange("p (j c) -> p j c", c=65)

        def rope_half(dst, raw, p, gain, hf, swname):
            # dst = gain * (raw * cos + pairswap(raw) * sin); rq tables carry
            # the 1/sqrt(hd) scale for q, k compensates with gain=8.
            # Only the final add writes dst (the matmul-consumed tile) so it
            # alone carries the f32r-rounded output annotation.
            cs = ds(1024 * hf, 1024)
            cosap = rq_sb[0:p, 1024 * hf : 1024 * hf + 1024]
            sinap = rq_sb[0:p, 2048 + 1024 * hf : 2048 + 1024 * hf + 1024]
            sw = scpool.tile([128, 1024], FP, tag="sc", name=f"sw_{swname}{hf}")
            nc.vector.stream_shuffle(sw[0:p, :], raw[0:p, cs], SWAP_MASK)
            t0 = scpool.tile([128, 1024], FP, tag="sc", name=f"t0_{swname}{hf}")
            nc.vector.scalar_tensor_tensor(t0[0:p, :], raw[0:p, cs], gain, cosap, MUL, MUL)
            # raw is dead after t0; reuse it as the second temp
            nc.vector.scalar_tensor_tensor(raw[0:p, cs], sw[0:p, :], gain, sinap, MUL, MUL)
            nc.vector.tensor_add(f32r(dst[0:p, cs]), t0[0:p, :], raw[0:p, cs])

        for hf in range(2):  # column halves (1024 qs/ks positions each)
            pt = [[psA.tile([128, 512], FP, tag="pj", name=f"pj_{hf}_{m}_{n2}") for n2 in range(2)] for m in range(3)]
            for g in range(16):
                hc = hpool.tile([128, 1024], FP, tag="hc")
                nc.sync.dma_start(f32r(hc[:]), hT[:, ds(2048 * g + 1024 * hf, 1024)])
                if hf == 0 and g == 0:
                    for k in range(1, 4):
                        nc.sync.dma_start(
                            f32r(wqt[k][:]), wq[:, ds(4 * 384 * k, 4 * 384)]
                        )
                for m in range(3):
                    for n2 in range(2):
                        nc.tensor.matmul(
                            pt[m][n2][:],
                            f32r(wq_ap(g, m)),
                            f32r(hc[:, ds(512 * n2, 512)]),
                            start=(g == 0),
                            stop=(g == 15),
                        )
            for m in range(3):
                dst = (q01, q23, kv)[m]
                for n2 in range(2):
                    nc.scalar.copy(dst[:, ds(1024 * hf + 512 * n2, 512)], pt[m][n2][:])

            if hf == 0:
                # deferred const DMAs: keep the first hT half uncontended
                nc.sync.dma_start(rq_sb[:], rq)
                nc.sync.dma_start(f32r(tri_sb[:]), tri)
                nc.sync.dma_start(f32r(wo_sb[:]), wo)
                # ones column for the softmax denominator
                nc.vector.tensor_copy(
                    f32r(v65[:, :, 64:65]),
                    tri_sb[:, 127:128][:, None, :].to_broadcast([128, 16, 1]),
                )

            rope_half(q01s, q01, 128, 1.0, hf, "q01")
            rope_half(q23s, q23, 128, 1.0, hf, "q23")
            rope_half(ks, kv, 64, 8.0, hf, "k")
            # duplicate rotated k at partitions 64-127: odd heads' score
            # matmuls then read lhsT/rhs both at base 64 (row-tiled pair)
            nc.sync.dma_start(
                f32r(ks[64:128, ds(1024 * hf, 1024)]),
                f32r(ks[0:64, ds(1024 * hf, 1024)]),
            )
            for b in range(2 * hf, 2 * hf + 2):
                vt = psT.tile([128, 256], FP, tag="vt", name=f"vt_{b}")
                for q in range(4):
                    j = 4 * b + q
                    nc.tensor.transpose(
                        vt[:, ds(64 * q, 64)],
                        kv[64:128, ds(128 * j, 128)],
                        ident64,
                    )
                nc.vector.tensor_copy(
                    f32r(v65[:, 4 * b : 4 * b + 4, 0:64]),
                    vt[:].rearrange("p (q c) -> p q c", c=64),
                )

        # release projection-phase SBUF/PSUM before the attention pools open
        phase1_ctx.close()
        expool = ctx.enter_context(tc.tile_pool(name="ex", bufs=4))
        rspool = ctx.enter_context(tc.tile_pool(name="rs", bufs=2))
        rbpool = ctx.enter_context(tc.tile_pool(name="rb", bufs=2))
        nmpool = ctx.enter_context(tc.tile_pool(name="nm", bufs=2))
        psS = ctx.enter_context(tc.tile_pool(name="psS", bufs=2, space="PSUM"))
        psO = ctx.enter_context(tc.tile_pool(name="psO", bufs=1, space="PSUM"))
        psP = ctx.enter_context(tc.tile_pool(name="psP", bufs=2, space="PSUM"))

        # ---- phase 2+3: attention + out-projection per qs-chunk --------
        for c in range(4):
            nj = 4 * c + 4  # number of live ks blocks for this qs chunk
            for hp in range(2):  # head pairs
                po = psO.tile([65, 1024], FP, tag="po")
                for j in range(nj):
                    r = j - 4 * c  # >= 0 on diagonal blocks
                    ps = psS.tile([128, 1024], FP, tag="ps")
                    for hh in range(2):
                        h = 2 * hp + hh
                        qt = q01s if h < 2 else q23s
                        base = 64 * (h % 2)
                        nc.tensor.matmul(
                            ps[:, ds(512 * hh, 512)],
                            f32r(ks[base : base + 64, ds(128 * j, 128)]),
                            f32r(qt[base : base + 64, ds(512 * c, 512)]),
                        )
                    ex = expool.tile([128, 1024], FP, tag="ex")
                    if r < 0:
                        nc.scalar.activation(f32r(ex[:]), ps[:], EXP)
                        off = 0
                    else:
                        off = 128 * r
                        w = 512 - off
                        psv = ps.rearrange("p (h w) -> p h w", w=512)[:, :, ds(off, w)]
                        exv = ex.rearrange("p (h w) -> p h w", w=512)[:, :, ds(off, w)]
                        nc.scalar.activation(f32r(exv), psv, EXP)
                        exd = ex.rearrange("p (h w) -> p h w", w=512)[:, :, ds(off, 128)]
                        nc.vector.tensor_mul(
                            f32r(exd),
                            exd,
                            tri_sb[:, 0:128][:, None, :].to_broadcast([128, 2, 128]),
                        )
                    for hh in range(2):
                        nc.tensor.matmul(
                            po[0:65, ds(512 * hh + off, 512 - off)],
                            f32r(v_sb[:, ds(65 * j, 65)]),
                            f32r(ex[:, ds(512 * hh + off, 512 - off)]),
                            start=(j == 0),
                            stop=(j == nj - 1),
                            skip_group_check=True,
                        )
                # copy the whole accumulator psum->sbuf in one op so the
                # banks free for the next head pair, then normalize from
                # SBUF: ones-column matmul broadcast of the sums row,
                # reciprocal on 64 lanes, two multiplies
                poc = rspool.tile([65, 1024], FP, tag="rs", name=f"poc_{c}_{hp}")
                nc.vector.tensor_copy(f32r(poc[:, :]), po[:, :])
                rbr = rbpool.tile([64, 1024], FP, tag="rbr")
                for half in range(2):
                    rbp = psP.tile(
                        [64, 512], FP, tag="pp", name=f"rbp_{c}_{hp}_{half}"
                    )
                    nc.tensor.matmul(
                        rbp[:], f32r(ones_row), f32r(poc[64:65, ds(512 * half, 512)])
                    )
                    nc.vector.reciprocal(rbr[0:64, ds(512 * half, 512)], rbp[0:64, :])
                dsttile = o2a if hp == 0 else o2b
                # hh=0 writes base 0 directly; hh=1 must land at partitions
                # 64-127, which DVE lanes can't write from base-0 inputs --
                # bounce through a base-0 scratch and DMA across partitions
                nc.vector.tensor_mul(
                    f32r(dsttile[0:64, ds(512 * c, 512)]),
                    poc[0:64, ds(0, 512)],
                    rbr[0:64, ds(0, 512)],
                )
                nm = nmpool.tile([64, 512], FP, tag="nm")
                nc.vector.tensor_mul(
                    f32r(nm[0:64, :]), poc[0:64, ds(512, 512)], rbr[0:64, ds(512, 512)]
                )
                nc.sync.dma_start(f32r(dsttile[64:128, ds(512 * c, 512)]), f32r(nm[0:64, :]))
            for b in range(4):
                for n in range(4):
                    pp = psP.tile([128, 512], FP, tag="pp", name=f"pp_{c}_{b}_{n}")
                    nc.tensor.matmul(
                        pp[:],
                        f32r(o2a[:, ds(512 * c + 128 * b, 128)]),
                        f32r(wo_sb[:, ds(512 * n, 512)]),
                        start=True,
                        stop=False,
                    )
                    nc.tensor.matmul(
                        pp[:],
                        f32r(o2b[:, ds(512 * c + 128 * b, 128)]),
                        f32r(wo_sb[:, ds(2048 + 512 * n, 512)]),
                        start=False,
                        stop=True,
                    )
                    st = outpool.tile([128, 512], FP, tag="st")
                    nc.vector.tensor_copy(st[:], pp[:])
                    nc.sync.dma_start(
                        out[ds(128 * (4 * c + b), 128), ds(512 * n, 512)], st[:]
                    )

    nc.compile()
    return nc


def get_module():
    if "nc" not in _CACHE:
        _CACHE["nc"] = _build_module()
    return _CACHE["nc"]


def _pack16(x):
    # [16*128, N] -> [128, 16*N] with [p, N*g + n] = x[128*g + p, n]
    n = x.shape[1]
    return (
        np.ascontiguousarray(x.reshape(16, 128, n).transpose(1, 0, 2)).reshape(128, 16 * n)
    )


def prep_inputs(hidden_states, freqs_cis, wqkv, wo):
    h = np.asarray(hidden_states, dtype=np.float32)[0]  # [S, D]
    fc = np.asarray(freqs_cis, dtype=np.float32)  # [S, 32, 2]
    wqkv = np.asarray(wqkv, dtype=np.float32)  # [3072, D]
    wo = np.asarray(wo, dtype=np.float32)  # [D, D]

    hT_sb = _pack16(np.ascontiguousarray(h.T))  # [128, 16*2048]

    cos = fc[:, :, 0]  # [S, 32]
    sin = fc[:, :, 1]
    cos_ext = np.repeat(cos, 2, axis=1).T  # [64, S], cos_ext[d, s] = cos(s, d//2)
    sgn = np.where(np.arange(HD) % 2 == 0, -1.0, 1.0).astype(np.float32)[:, None]
    sin_ext = np.repeat(sin, 2, axis=1).T * sgn  # sin'[d, s]
    scale = 1.0 / np.sqrt(np.float32(HD))
    rq_np = np.concatenate(
        [np.tile(cos_ext * scale, (2, 1)), np.tile(sin_ext * scale, (2, 1))], axis=1
    ).astype(np.float32)  # [128, 4096]
    tri_np = np.concatenate(
        [
            (np.arange(128)[:, None] <= np.arange(128)[None, :]).astype(np.float32),
            np.eye(128, dtype=np.float32),
        ],
        axis=1,
    )  # [128, 256]: triangle | identity

    in_maps = []
    for i in range(NCORES):
        wl = np.concatenate(
            [
                wqkv[256 * i : 256 * i + 256],
                wqkv[D + 64 * i : D + 64 * i + 64],
                wqkv[D + KV_SIZE + 64 * i : D + KV_SIZE + 64 * i + 64],
            ],
            axis=0,
        )  # [384, D]
        wq_sb = _pack16(np.ascontiguousarray(wl.T))  # [128, 16*384]
        woT = np.ascontiguousarray(wo[:, 256 * i : 256 * i + 256].T)  # [256, D]
        wo_sb = np.ascontiguousarray(
            woT.reshape(2, 128, D).transpose(1, 0, 2)
        ).reshape(128, 2 * D)
        in_maps.append(
            {
                "hT": hT_sb,
                "wq": wq_sb,
                "wo": wo_sb,
                "rq": rq_np,
                "tri": tri_np,
            }
        )
    return in_maps


def run_on_hw(in_maps, trace=False, **kw):
    from concourse.bass_utils import run_bass_kernel_spmd

    nc = get_module()
    return run_bass_kernel_spmd(nc, in_maps, list(range(NCORES)), trace=trace, **kw)


def kernel(hidden_states, freqs_cis, wqkv, wo):
    in_maps = prep_inputs(hidden_states, freqs_cis, wqkv, wo)
    res = run_on_hw(in_maps)
    acc = np.zeros((S, D), dtype=np.float64)
    for r in res.results:
        acc += r["out"]
    return acc.astype(np.float32).reshape(1, S, D)


# revision 48
# speedup vs baseline: 1.0211x; 1.0211x over previous
"""Tensor-parallel (over GQA head groups) multi-head attention for 8 trn2 cores.

Each core owns 4 query heads + their shared kv head (one GQA group), the
matching 384 rows of wqkv and 256 columns of wo.  Every core computes a full
[S, D] partial of the output projection; the host sums the 8 partials.

Dataflow inside one core (fp32 data, float32r PE mode for all big matmuls --
same bytes, 4x the fp32 matmul rate at ~3e-4 relative error):
  qkvT [384, S] = wqkvT_local.T @ hT          (contraction dim on partitions)
  rope(qT, kT)  = raw*cos + pairswap(raw)*sin (pair swap via the DVE 32-lane
                                               stream_shuffle; rotation sign
                                               folded into the sin table,
                                               1/sqrt(hd) into the q tables)
  sT [ks, qs]   = kT.T-slices @ qT            (scores transposed so the
                                               softmax denominator can come
                                               from a matmul: even/odd heads
                                               at base partitions 0/64 run as
                                               concurrent row-tiled matmuls)
  exp on ScalarE straight out of PSUM, no max-subtraction (scores ~N(0,1));
  causal masking = skip fully-masked column ranges + one triangle multiply
  per diagonal block.
  out2T [65, qs] += v_aug.T @ exp             (ones column appended to V:
                                               row 64 = softmax denominator)
  o2 = out2T[0:64] * bcast(1/out2T[64])       (broadcast via ones-column
                                               matmul; reciprocal on 64 lanes)
  partial [qs, e] = o2-slices.T @ woT_local   (accumulate 2 contraction chunks)
"""

import sys

if "/opt/trn_rl_repo" not in sys.path:
    sys.path.insert(0, "/opt/trn_rl_repo")

import numpy as np

S = 2048
D = 2048
HD = 64
N_HEAD = 32
N_KV = 8
NCORES = 8
QH_PER_CORE = N_HEAD // NCORES  # 4
KV_SIZE = N_KV * HD  # 512

_CACHE = {}


def _build_module():
    from contextlib import ExitStack

    import concourse.mybir as mybir
    import concourse.tile as tile
    from concourse import bacc
    from concourse.bass import ds

    FP = mybir.dt.float32
    FPR = mybir.dt.float32r  # fp32 bits, single-pass PE mode: 4x matmul rate
    EXP = mybir.ActivationFunctionType.Exp

    def f32r(ap):
        return ap.bitcast(FPR)

    nc = bacc.Bacc(
        "TRN2",
        target_bir_lowering=False,
        debug=False,
        enable_asserts=False,
        num_devices=NCORES,
    )

    # [p, 2048*g + s] = hidden[s, 128*g + p]
    hT = nc.dram_tensor("hT", [128, 16 * S], FPR, kind="ExternalInput").ap()
    # [p, 384*g + r] = wqkv_local[r, 128*g + p]; r: 0-255 q, 256-319 k, 320-383 v
    wq = nc.dram_tensor("wq", [128, 16 * 384], FPR, kind="ExternalInput").ap()
    # [p, 2048*g + e] = wo[e, 256*core + 128*g + p]
    wo = nc.dram_tensor("wo", [128, 2 * 2048], FPR, kind="ExternalInput").ap()
    # [p, s] cos (cols 0:2048) | sin' (cols 2048:4096), two head copies, /8
    rq = nc.dram_tensor("rq", [128, 2 * S], FP, kind="ExternalInput").ap()
    # cols 0:128 tri[p, f] = (p <= f); cols 128:256 identity[p, f] = (p == f)
    tri = nc.dram_tensor("tri", [128, 256], FPR, kind="ExternalInput").ap()
    out = nc.dram_tensor("out", [S, D], FP, kind="ExternalOutput").ap()

    with tile.TileContext(nc) as tc, ExitStack() as ctx:
        const = ctx.enter_context(tc.tile_pool(name="const", bufs=1))
        # wq as 4 separate tiles; chunk 0 DMAs now, chunks 1-3 interleave
        # into the first g-loop so the opening hT chunks aren't starved
        wqt = [
            const.tile([128, 4 * 384], FP, tag=f"wq{k}", name=f"wq_sb{k}")
            for k in range(4)
        ]
        nc.sync.dma_start(f32r(wqt[0][:]), wq[:, ds(0, 4 * 384)])

        def wq_ap(g, m):
            return wqt[g // 4][:, ds(384 * (g % 4) + 128 * m, 128)]

        wo_sb = const.tile([128, 4096], FP, tag="wo")
        rq_sb = const.tile([128, 4096], FP, tag="rq")
        tri_sb = const.tile([128, 256], FP, tag="tri")
        # identity slice at partitions 64-127: the v-transpose matmul's
        # operands then share base_partition 64 (v lives in kv[64:128])
        ident64 = tri_sb[64:128, 192:256]
        # all-ones row hiding inside the triangle mask: tri[64, 64:128] == 1
        ones_row = tri_sb[64:65, 64:128]

        qraw = ctx.enter_context(tc.tile_pool(name="qraw", bufs=1))
        q01 = qraw.tile([128, S], FP, tag="q01")
        q23 = qraw.tile([128, S], FP, tag="q23")
        kv = qraw.tile([128, S], FP, tag="kv")
        qsw = ctx.enter_context(tc.tile_pool(name="qsw", bufs=1))
        q01s = qsw.tile([128, S], FP, tag="q01s")
        q23s = qsw.tile([128, S], FP, tag="q23s")
        ks = qsw.tile([128, S], FP, tag="ks")
        vpool = ctx.enter_context(tc.tile_pool(name="vsb", bufs=1))
        v_sb = vpool.tile([128, 16 * 65], FP, tag="v")
        o2pool = ctx.enter_context(tc.tile_pool(name="o2", bufs=1))
        o2a = o2pool.tile([128, S], FP, tag="o2a")
        o2b = o2pool.tile([128, S], FP, tag="o2b")

        outpool = ctx.enter_context(tc.tile_pool(name="ost", bufs=4))

        phase1_ctx = ExitStack()
        hpool = phase1_ctx.enter_context(tc.tile_pool(name="hp", bufs=6))
        scpool = phase1_ctx.enter_context(tc.tile_pool(name="sc", bufs=2))
        psA = phase1_ctx.enter_context(tc.tile_pool(name="psA", bufs=6, space="PSUM"))
        psT = phase1_ctx.enter_context(tc.tile_pool(name="psT", bufs=2, space="PSUM"))

        # ---- phase 1 + 1.5: qkvT projection, rope + v transpose --------
        # interleaved per 1024-wide column half so attention on qs-chunks
        # 0-1 can start while the second half is still projecting
        SWAP_MASK = [i ^ 1 for i in range(32)]
        MUL = mybir.AluOpType.mult
        v65 = v_sb.rearrange("p (j c) -> p j c", c=65)

        def rope_half(dst, raw, p, gain, hf, swname):
            # dst = gain * (raw * cos + pairswap(raw) * sin); rq tables carry
            # the 1/sqrt(hd) scale for q, k compensates with gain=8.
            # Only the final add writes dst (the matmul-consumed tile) so it
            # alone carries the f32r-rounded output annotation.
            cs = ds(1024 * hf, 1024)
            cosap = rq_sb[0:p, 1024 * hf : 1024 * hf + 1024]
            sinap = rq_sb[0:p, 2048 + 1024 * hf : 2048 + 1024 * hf + 1024]
            sw = scpool.tile([128, 1024], FP, tag="sc", name=f"sw_{swname}{hf}")
            nc.vector.stream_shuffle(sw[0:p, :], raw[0:p, cs], SWAP_MASK)
            t0 = scpool.tile([128, 1024], FP, tag="sc", name=f"t0_{swname}{hf}")
            nc.vector.scalar_tensor_tensor(t0[0:p, :], raw[0:p, cs], gain, cosap, MUL, MUL)
            # raw is dead after t0; reuse it as the second temp
            nc.vector.scalar_tensor_tensor(raw[0:p, cs], sw[0:p, :], gain, sinap, MUL, MUL)
            nc.vector.tensor_add(f32r(dst[0:p, cs]), t0[0:p, :], raw[0:p, cs])

        for hf in range(2):  # column halves (1024 qs/ks positions each)
            pt = [[psA.tile([128, 512], FP, tag="pj", name=f"pj_{hf}_{m}_{n2}") for n2 in range(2)] for m in range(3)]
            for g in range(16):
                hc = hpool.tile([128, 1024], FP, tag="hc")
                nc.sync.dma_start(f32r(hc[:]), hT[:, ds(2048 * g + 1024 * hf, 1024)])
                if hf == 0 and g == 0:
                    for k in range(1, 4):
                        nc.sync.dma_start(
                            f32r(wqt[k][:]), wq[:, ds(4 * 384 * k, 4 * 384)]
                        )
                for m in range(3):
                    for n2 in range(2):
                        nc.tensor.matmul(
                            pt[m][n2][:],
                            f32r(wq_ap(g, m)),
                            f32r(hc[:, ds(512 * n2, 512)]),
                            start=(g == 0),
                            stop=(g == 15),
                        )
            for m in range(3):
                dst = (q01, q23, kv)[m]
                for n2 in range(2):
                    nc.scalar.copy(dst[:, ds(1024 * hf + 512 * n2, 512)], pt[m][n2][:])

            if hf == 0:
                # deferred const DMAs: keep the first hT half uncontended
                nc.sync.dma_start(rq_sb[:], rq)
                nc.sync.dma_start(f32r(tri_sb[:]), tri)
                nc.sync.dma_start(f32r(wo_sb[:]), wo)
                # ones column for the softmax denominator
                nc.vector.tensor_copy(
                    f32r(v65[:, :, 64:65]),
                    tri_sb[:, 127:128][:, None, :].to_broadcast([128, 16, 1]),
                )

            rope_half(q01s, q01, 128, 1.0, hf, "q01")
            rope_half(q23s, q23, 128, 1.0, hf, "q23")
            rope_half(ks, kv, 64, 8.0, hf, "k")
            # duplicate rotated k at partitions 64-127: odd heads' score
            # matmuls then read lhsT/rhs both at base 64 (row-tiled pair)
            nc.sync.dma_start(
                f32r(ks[64:128, ds(1024 * hf, 1024)]),
                f32r(ks[0:64, ds(1024 * hf, 1024)]),
            )
            for b in range(2 * hf, 2 * hf + 2):
                vt = psT.tile([128, 256], FP, tag="vt", name=f"vt_{b}")
                for q in range(4):
                    j = 4 * b + q
                    nc.tensor.transpose(
                        vt[:, ds(64 * q, 64)],
                        kv[64:128, ds(128 * j, 128)],
                        ident64,
                    )
                nc.vector.tensor_copy(
                    f32r(v65[:, 4 * b : 4 * b + 4, 0:64]),
                    vt[:].rearrange("p (q c) -> p q c", c=64),
                )

        # release projection-phase SBUF/PSUM before the attention pools open
        phase1_ctx.close()
        expool = ctx.enter_context(tc.tile_pool(name="ex", bufs=4))
        rspool = ctx.enter_context(tc.tile_pool(name="rs", bufs=2))
        rbpool = ctx.enter_context(tc.tile_pool(name="rb", bufs=2))
        nmpool = ctx.enter_context(tc.tile_pool(name="nm", bufs=2))
        psS = ctx.enter_context(tc.tile_pool(name="psS", bufs=2, space="PSUM"))
        psO = ctx.enter_context(tc.tile_pool(name="psO", bufs=1, space="PSUM"))
        psP = ctx.enter_context(tc.tile_pool(name="psP", bufs=2, space="PSUM"))

        # ---- phase 2+3: attention + out-projection per qs-chunk --------
        for c in range(4):
            nj = 4 * c + 4  # number of live ks blocks for this qs chunk
            for hp in range(2):  # head pairs
                po = psO.tile([65, 1024], FP, tag="po")
                for j in range(nj):
                    r = j - 4 * c  # >= 0 on diagonal blocks
                    ps = psS.tile([128, 1024], FP, tag="ps")
                    for hh in range(2):
                        h = 2 * hp + hh
                        qt = q01s if h < 2 else q23s
                        base = 64 * (h % 2)
                        nc.tensor.matmul(
                            ps[:, ds(512 * hh, 512)],
                            f32r(ks[base : base + 64, ds(128 * j, 128)]),
                            f32r(qt[base : base + 64, ds(512 * c, 512)]),
                        )
                    ex = expool.tile([128, 1024], FP, tag="ex")
                    if r < 0:
                        nc.scalar.activation(f32r(ex[:]), ps[:], EXP)
                        off = 0
                    else:
                        off = 128 * r
                        w = 512 - off
                        psv = ps.rearrange("p (h w) -> p h w", w=512)[:, :, ds(off, w)]
                        exv = ex.rearrange("p (h w) -> p h w", w=512)[:, :, ds(off, w)]
                        nc.scalar.activation(f32r(exv), psv, EXP)
                        exd = ex.rearrange("p (h w) -> p h w", w=512)[:, :, ds(off, 128)]
                        nc.vector.tensor_mul(
                            f32r(exd),
                            exd,
                            tri_sb[:, 0:128][:, None, :].to_broadcast([128, 2, 128]),
                        )
                    for hh in range(2):
                        nc.tensor.matmul(
                            po[0:65, ds(512 * hh + off, 512 - off)],
                            f32r(v_sb[:, ds(65 * j, 65)]),
                            f32r(ex[:, ds(512 * hh + off, 512 - off)]),
                            start=(j == 0),
                            stop=(j == nj - 1),
                            skip_group_check=True,
                        )
                # copy the whole accumulator psum->sbuf in one op so the
                # banks free for the next head pair, then normalize from
                # SBUF: ones-column matmul broadcast of the sums row,
                # reciprocal on 64 lanes, two multiplies
                poc = rspool.tile([65, 1024], FP, tag="rs", name=f"poc_{c}_{hp}")
                nc.vector.tensor_copy(f32r(poc[:, :]), po[:, :])
                rbr = rbpool.tile([64, 1024], FP, tag="rbr")
                for half in range(2):
                    rbp = psP.tile(
                        [64, 512], FP, tag="pp", name=f"rbp_{c}_{hp}_{half}"
                    )
                    nc.tensor.matmul(
                        rbp[:], f32r(ones_row), f32r(poc[64:65, ds(512 * half, 512)])
                    )
                    nc.vector.reciprocal(rbr[0:64, ds(512 * half, 512)], rbp[0:64, :])
                dsttile = o2a if hp == 0 else o2b
                # hh=0 writes base 0 directly; hh=1 must land at partitions
                # 64-127, which DVE lanes can't write from base-0 inputs --
                # bounce through a base-0 scratch and DMA across partitions
                nc.vector.tensor_mul(
                    f32r(dsttile[0:64, ds(512 * c, 512)]),
                    poc[0:64, ds(0, 512)],
                    rbr[0:64, ds(0, 512)],
                )
                nm = nmpool.tile([64, 512], FP, tag="nm")
                nc.vector.tensor_mul(
                    f32r(nm[0:64, :]), poc[0:64, ds(512, 512)], rbr[0:64, ds(512, 512)]
                )
                nc.sync.dma_start(f32r(dsttile[64:128, ds(512 * c, 512)]), f32r(nm[0:64, :]))
            for b in range(4):
                for n2 in range(2):  # pairs of 512-wide e-slices -> one DMA
                    st = outpool.tile([128, 1024], FP, tag="st", name=f"st_{c}_{b}_{n2}")
                    for nn in range(2):
                        n = 2 * n2 + nn
                        pp = psP.tile([128, 512], FP, tag="pp", name=f"pp_{c}_{b}_{n}")
                        nc.tensor.matmul(
                            pp[:],
                            f32r(o2a[:, ds(512 * c + 128 * b, 128)]),
                            f32r(wo_sb[:, ds(512 * n, 512)]),
                            start=True,
                            stop=False,
                        )
                        nc.tensor.matmul(
                            pp[:],
                            f32r(o2b[:, ds(512 * c + 128 * b, 128)]),
                            f32r(wo_sb[:, ds(2048 + 512 * n, 512)]),
                            start=False,
                            stop=True,
                        )
                        nc.vector.tensor_copy(st[:, ds(512 * nn, 512)], pp[:])
                    nc.sync.dma_start(
                        out[ds(128 * (4 * c + b), 128), ds(1024 * n2, 1024)], st[:]
                    )

    nc.compile()
    return nc


def get_module():
    if "nc" not in _CACHE:
        _CACHE["nc"] = _build_module()
    return _CACHE["nc"]


def _pack16(x):
    # [16*128, N] -> [128, 16*N] with [p, N*g + n] = x[128*g + p, n]
    n = x.shape[1]
    return (
        np.ascontiguousarray(x.reshape(16, 128, n).transpose(1, 0, 2)).reshape(128, 16 * n)
    )


def prep_inputs(hidden_states, freqs_cis, wqkv, wo):
    h = np.asarray(hidden_states, dtype=np.float32)[0]  # [S, D]
    fc = np.asarray(freqs_cis, dtype=np.float32)  # [S, 32, 2]
    wqkv = np.asarray(wqkv, dtype=np.float32)  # [3072, D]
    wo = np.asarray(wo, dtype=np.float32)  # [D, D]

    hT_sb = _pack16(np.ascontiguousarray(h.T))  # [128, 16*2048]

    cos = fc[:, :, 0]  # [S, 32]
    sin = fc[:, :, 1]
    cos_ext = np.repeat(cos, 2, axis=1).T  # [64, S], cos_ext[d, s] = cos(s, d//2)
    sgn = np.where(np.arange(HD) % 2 == 0, -1.0, 1.0).astype(np.float32)[:, None]
    sin_ext = np.repeat(sin, 2, axis=1).T * sgn  # sin'[d, s]
    scale = 1.0 / np.sqrt(np.float32(HD))
    rq_np = np.concatenate(
        [np.tile(cos_ext * scale, (2, 1)), np.tile(sin_ext * scale, (2, 1))], axis=1
    ).astype(np.float32)  # [128, 4096]
    tri_np = np.concatenate(
        [
            (np.arange(128)[:, None] <= np.arange(128)[None, :]).astype(np.float32),
            np.eye(128, dtype=np.float32),
        ],
        axis=1,
    )  # [128, 256]: triangle | identity

    in_maps = []
    for i in range(NCORES):
        wl = np.concatenate(
            [
                wqkv[256 * i : 256 * i + 256],
                wqkv[D + 64 * i : D + 64 * i + 64],
                wqkv[D + KV_SIZE + 64 * i : D + KV_SIZE + 64 * i + 64],
            ],
            axis=0,
        )  # [384, D]
        wq_sb = _pack16(np.ascontiguousarray(wl.T))  # [128, 16*384]
        woT = np.ascontiguousarray(wo[:, 256 * i : 256 * i + 256].T)  # [256, D]
        wo_sb = np.ascontiguousarray(
            woT.reshape(2, 128, D).transpose(1, 0, 2)
        ).reshape(128, 2 * D)
        in_maps.append(
            {
                "hT": hT_sb,
                "wq": wq_sb,
                "wo": wo_sb,
                "rq": rq_np,
                "tri": tri_np,
            }
        )
    return in_maps


def run_on_hw(in_maps, trace=False, **kw):
    from concourse.bass_utils import run_bass_kernel_spmd

    nc = get_module()
    return run_bass_kernel_spmd(nc, in_maps, list(range(NCORES)), trace=trace, **kw)


def kernel(hidden_states, freqs_cis, wqkv, wo):
    in_maps = prep_inputs(hidden_states, freqs_cis, wqkv, wo)
    res = run_on_hw(in_maps)
    acc = np.zeros((S, D), dtype=np.float64)
    for r in res.results:
        acc += r["out"]
    return acc.astype(np.float32).reshape(1, S, D)
